# revision 20
# baseline (speedup 1.0000x reference)
"""Box-attention kernel for Trainium2 (Bass/Tile), SPMD over 8 NeuronCores.

Problem: per-(batch, h, w) pixel attention over 32 boxes:
  S[i,j] = <q[i,:,p], k[j,:,p]>/8 ; W = softmax_j S ; delta[i,c,p] = sum_j W[i,j] v[j,c,p]

Sharding: core = 2*b + h_half; each core owns (b = core//2, h in [40*(core%2), +40)).
Zero communication (pixel-parallel).

v2 design (vs v1 baseline at 1.245 ms):
 - fp16 I/O: host casts q,k,v to fp16 and the output back to fp32. Halves HBM
   traffic (52.4 MB/core); fp16 error (2^-11) is far inside the 2e-2 gate.
 - chunk = 8 h-rows; q/k SBUF [64*(hc//4)+c][i, hc%4, w] -> 640B DMA lines;
   v/out [32*(hc//2)+{j,i}][c, hc%2, w] -> 320B lines.
 - compact score banks: e-bank [128, 512] holds 64 pixels (4 part-groups x
   16 col-groups of 32) with zero padding waste. One exp per 64 px.
 - denominator fused into the delta matmul: v tile has a 65th channel == 1.0,
   so out[.., 64] = sum_j E[j,i] = denom. No ones-matmul, no [128,512]
   reciprocal. Reciprocal runs on [128,16] compact denominators instead
   (DVE iterative divide is ~6.5 cyc/elem -- was 53% of v1's span).
 - normalize+evacuate fused: one DVE tensor_mult per ob bank with rden
   broadcast (step-0) along c, writing fp16 straight into the staging tile.
 - PSUM: eb x2, (ob0,ob1,ob2) x2 = 8 banks exactly.
 - emission skew: scores(n+1) issued before delta(n) so the PE never waits
   on the ACT exp; engines pipeline across supergroups.
"""

import sys

import numpy as np

try:
    import concourse.bass as bass
except ImportError:  # fresh grading dir: point at the in-container repo
    for p in ("/opt/trn_rl_repo", "/root/.axon_site/_ro/trn_rl_repo"):
        if p not in sys.path:
            sys.path.insert(0, p)
    import concourse.bass as bass

from contextlib import ExitStack

import concourse.bacc as bacc
import concourse.tile as tile
from concourse import mybir
from concourse.bass_utils import run_bass_kernel_spmd

NB, B, C, H, W = 32, 4, 64, 80, 80
HPC = H // 2  # h rows per core (8 cores = 4 batches x 2 h-halves)
CHH = 8  # chunk height (h rows)
NCHUNK = HPC // CHH  # 5
WB = 16  # w values per supergroup (64 px = 4 groups x 16 cols)
NSGW = W // WB  # 5 supergroups per (chunk, r)
F16 = mybir.dt.float16
F32 = mybir.dt.float32

_CACHE = {}


def build_nc():
    # The host pre-permutes inputs into the exact SBUF layouts (see _pack_*),
    # so every DMA is [128 partitions x 20KB contiguous] - descriptor-perfect.
    # q/k: [hb][64*(hc//4)+c][i, (hc%4)*80+w]; v: [hb][32*(hc//2)+j][c, (hc%2)*80+w]
    # o (output) mirrors v with i instead of j.
    nc = bacc.Bacc()
    q = nc.declare_dram_parameter("q", [NCHUNK, 128, NB, 4 * W], F16, isOutput=False)
    k = nc.declare_dram_parameter("k", [NCHUNK, 128, NB, 4 * W], F16, isOutput=False)
    v = nc.declare_dram_parameter("v", [NCHUNK, 128, C, 2 * W], F16, isOutput=False)
    o = nc.declare_dram_parameter("o", [NCHUNK, 128, C, 2 * W], F16, isOutput=True)
    qv, kv, vv, ov = q[:], k[:], v[:], o[:]

    with tile.TileContext(nc) as tc, ExitStack() as ctx:
        io = ctx.enter_context(tc.tile_pool(name="io", bufs=2))
        sm = ctx.enter_context(tc.tile_pool(name="sm", bufs=3))
        ep = ctx.enter_context(tc.tile_pool(name="ep", bufs=2, space="PSUM"))
        op = ctx.enter_context(tc.tile_pool(name="op", bufs=2, space="PSUM"))

        chunk_tiles = {}

        def emit_chunk_dma(hb):
            tq = io.tile([128, 32, 4, W], F16, tag="tq")
            tk = io.tile([128, 32, 4, W], F16, tag="tk")
            tv = io.tile([128, 65, 2, W], F16, tag="tv")
            to = io.tile([128, C, 2, W], F16, tag="to")
            tqf = tq.rearrange("p i hw w -> p i (hw w)")
            tkf = tk.rearrange("p j hw w -> p j (hw w)")
            tvf = tv.rearrange("p c r w -> p c (r w)")
            nc.sync.dma_start(out=tqf, in_=qv[hb])
            nc.sync.dma_start(out=tkf, in_=kv[hb])
            nc.sync.dma_start(out=tvf[:, 0:64], in_=vv[hb])
            nc.gpsimd.memset(tv[:, 64], 1.0)
            chunk_tiles[hb] = (tq, tk, tv, to)

        def emit_chunk_out(hb):
            (_, _, _, to) = chunk_tiles.pop(hb)
            tof = to.rearrange("p c r w -> p c (r w)")
            nc.sync.dma_start(out=ov[hb], in_=tof)

        def emit_scores(sg):
            hb, r, wb = sg["hb"], sg["r"], sg["wb"]
            tq, tk, _, _ = chunk_tiles[hb]
            eb = ep.tile([128, 512], F32, tag="eb")
            ob2 = op.tile([128, 388], F32, tag="ob2")
            sg["ob2"] = ob2
            dummy_lhsT = tq.rearrange("p i hw w -> p (i hw) w")[:, 0:128, 0]
            dummy_rhs = tq.rearrange("p i hw w -> p (i hw w)")[:, 0:128]
            for u in range(WB):
                w_ = WB * wb + u
                for g in range(4):
                    s, hw = g // 2, 2 * (g % 2) + r
                    nc.tensor.matmul(
                        out=eb[32 * g : 32 * g + 32, 32 * u : 32 * u + 32],
                        lhsT=tk[64 * s : 64 * s + 64, :, hw, w_],
                        rhs=tq[64 * s : 64 * s + 64, :, hw, w_],
                        start=True,
                        stop=True,
                        tile_position=(64 * s, 32 * g),
                    )
                # full-array filler: masked-tile MMs earn no HAM activity
                # credit; one 128x128xN=128 MM per u-group keeps K=8/8.
                nc.tensor.matmul(
                    out=ob2[:, 132:260], lhsT=dummy_lhsT, rhs=dummy_rhs,
                    start=True, stop=True, tile_position=(0, 0),
                )
            sg["eb"] = eb

        def emit_softmax_delta(sg):
            hb, r, wb = sg["hb"], sg["r"], sg["wb"]
            _, _, tv, to = chunk_tiles[hb]
            eb = sg.pop("eb")
            E = sm.tile([128, 512], F16, tag="E")
            nc.scalar.activation(E, eb, mybir.ActivationFunctionType.Exp, scale=0.125)
            ob0 = op.tile([128, 462], F32, tag="ob0")
            ob1 = op.tile([128, 462], F32, tag="ob1")
            ob2 = sg.pop("ob2")
            obs = [ob0, ob1, ob2]
            tq = chunk_tiles[hb][0]
            dummy_lhsT = tq.rearrange("p i hw w -> p (i hw) w")[:, 0:128, 0]
            dummy_rhs = tq.rearrange("p i hw w -> p (i hw w)")[:, 0:128]
            for u in range(WB):
                w_ = WB * wb + u
                ob, uu = obs[min(u // 7, 2)], u - 7 * min(u // 7, 2)
                for g in range(4):
                    nc.tensor.matmul(
                        out=ob[32 * g : 32 * g + 32, 66 * uu : 66 * uu + 65],
                        lhsT=E[32 * g : 32 * g + 32, 32 * u : 32 * u + 32],
                        rhs=tv[32 * g : 32 * g + 32, :, r, w_],
                        start=True,
                        stop=True,
                        tile_position=(32 * g, 32 * g),
                    )
                nc.tensor.matmul(
                    out=ob2[:, 260:388], lhsT=dummy_lhsT, rhs=dummy_rhs,
                    start=True, stop=True, tile_position=(0, 0),
                )
            # normalize + evacuate: out inner dim is w (unit-stride fp16
            # writes); the strided side is the PSUM read (c step 1, u step 66).
            rden = sm.tile([128, 16], F32, tag="rden")
            for bi, nb in ((0, 7), (1, 7), (2, 2)):
                obv = obs[bi][:, 0 : 66 * nb].rearrange("p (u c) -> p c u", c=66)
                u0 = 7 * bi
                w0 = WB * wb + u0
                nc.vector.reciprocal(rden[:, u0 : u0 + nb], obv[:, 64])
                nc.vector.tensor_mul(
                    to[:, :, r, w0 : w0 + nb],
                    obv[:, 0:64],
                    rden[:, u0 : u0 + nb].unsqueeze(1).broadcast_to((128, 64, nb)),
                )

        # HAM warmup: the 64x32/32x65 masked-tile matmuls don't register as
        # PE activity for the HAM clock gate, and warm expires after ~3.4us
        # of idle. Burst ~3.5-6us of full-array matmuls right before each
        # chunk's compute: reading that chunk's tq orders the burst after the
        # q-DMA, so it fills the PE-idle window while k/v DMAs finish and the
        # scores start at 2.4 GHz.
        def emit_warm_burst(hb, n):
            tq = chunk_tiles[hb][0]
            lhsT = tq.rearrange("p i hw w -> p (i hw) w")[:, 0:128, 0]
            rhs = tq.rearrange("p i hw w -> p (i hw w)")[:, 0:512]
            web = ep.tile([128, 512], F32, tag="eb", name="web")
            for _ in range(n):
                nc.tensor.matmul(
                    out=web, lhsT=lhsT, rhs=rhs,
                    start=True, stop=True, tile_position=(0, 0),
                )

        sgs = [
            {"hb": hb, "r": r, "wb": wb}
            for hb in range(NCHUNK)
            for r in range(2)
            for wb in range(NSGW)
        ]
        emit_chunk_dma(0)
        pending = None
        for n, sg in enumerate(sgs):
            if sg["wb"] == 0 and sg["r"] == 0:
                emit_warm_burst(sg["hb"], 14 if sg["hb"] == 0 else 8)
                if sg["hb"] + 1 < NCHUNK:
                    emit_chunk_dma(sg["hb"] + 1)
            emit_scores(sg)
            if pending is not None:
                emit_softmax_delta(pending)
                if pending["wb"] == NSGW - 1 and pending["r"] == 1:
                    emit_chunk_out(pending["hb"])
            pending = sg
        emit_softmax_delta(pending)
        emit_chunk_out(pending["hb"])
    nc.compile()
    return nc


def _get_nc():
    if "nc" not in _CACHE:
        _CACHE["nc"] = build_nc()
    return _CACHE["nc"]


def _pack_qk(a):
    # [32, 64, 40, 80] fp32 -> [5, 128=(s c), 32 i, 320=(hw w)] fp16
    t = np.asarray(a).reshape(NB, C, NCHUNK, 2, 4, W).transpose(2, 3, 1, 0, 4, 5)
    return t.astype(np.float16).reshape(NCHUNK, 128, NB, 4 * W)


def _pack_v(a):
    # [32, 64, 40, 80] fp32 -> [5, 128=(g j), 64 c, 160=(r w)] fp16
    t = np.asarray(a).reshape(NB, C, NCHUNK, 4, 2, W).transpose(2, 3, 0, 1, 4, 5)
    return t.astype(np.float16).reshape(NCHUNK, 128, C, 2 * W)


def _unpack_o(oh):
    # [5, 128=(g i), 64 c, 160=(r w)] fp16 -> [32, 64, 40, 80] fp32
    t = oh.reshape(NCHUNK, 4, NB, C, 2, W).astype(np.float32)
    return t.transpose(2, 3, 0, 1, 4, 5).reshape(NB, C, HPC, W)


def kernel(q_big, k_big, v_big, **run_kwargs):
    nc = _get_nc()
    in_maps = []
    for core in range(8):
        b, h0 = core // 2, HPC * (core % 2)
        sl = np.s_[:, b, :, h0 : h0 + HPC, :]
        in_maps.append(
            {
                "q": _pack_qk(q_big[sl]),
                "k": _pack_qk(k_big[sl]),
                "v": _pack_v(v_big[sl]),
            }
        )
    res = run_bass_kernel_spmd(nc, in_maps, list(range(8)), **run_kwargs)
    out = np.empty((NB, B, C, H, W), np.float32)
    for core in range(8):
        b, h0 = core // 2, HPC * (core % 2)
        out[:, b, :, h0 : h0 + HPC, :] = _unpack_o(res.results[core]["o"])
    if run_kwargs:
        kernel.last_results = res
    return out


# revision 21
# speedup vs baseline: 2.2980x; 2.2980x over previous
"""Box-attention kernel for Trainium2 (Bass/Tile), SPMD over 8 NeuronCores.

Problem: per-(batch, h, w) pixel attention over 32 boxes:
  S[i,j] = <q[i,:,p], k[j,:,p]>/8 ; W = softmax_j S ; delta[i,c,p] = sum_j W[i,j] v[j,c,p]

Sharding: core = 2*b + h_half; each core owns (b = core//2, h in [40*(core%2), +40)).
Zero communication (pixel-parallel).

Design (v5; v1 baseline was 1.245 ms):
 - fp16 I/O: host casts q,k,v to fp16 and the output back to fp32. Halves HBM
   traffic (52.4 MB/core); fp16 error (2^-11) is far inside the 2e-2 gate.
 - host pre-permutes inputs into the exact SBUF layouts, so every DMA is
   [128 partitions x contiguous bytes] - descriptor-perfect.
 - fine-grained chunks: 8 h-rows x 16 w = 128 pixels = 2 supergroups per
   chunk, 25 chunks, ~0.5 MB DMAs, bufs=4 -> deep prefetch, no pipeline
   bubbles at chunk boundaries.
 - compact score banks: e-bank [128, 512] holds 64 pixels (4 partition
   groups x 16 col-groups of 32) with no padding waste; one exp per 64 px.
 - denominator fused into the delta matmul: v tile has a 65th channel == 1.0
   so out[.., 64] = sum_j E[j,i] = denom. Reciprocal runs on [128,16]
   compact denominators (DVE iterative divide is ~6.5 cyc/elem).
 - normalize+evacuate fused: one DVE tensor_mult per ob bank with rden
   broadcast (step-0) along c, fp16 out with unit-stride writes.
 - PSUM: eb x2, (ob0,ob1,ob2) x2 = 8 banks exactly.
 - emission skew: scores(n+1) issued before softmax/delta(n) so the PE is
   never waiting on the ACT exp; engines pipeline across supergroups.

Pixel/partition mapping within a chunk (hc = h row in chunk, 0..7):
 - q/k tiles: [64*(hc//4) + c][i, hc%4, w]  (s = hc//4 selects row half)
 - v/out tiles: [32*(hc//2) + {j,i}][c, hc%2, w]  (g = hc//2, r = hc%2)
 - supergroup r: pixels (g, u) with hc = 2g+r, w = u; score tile for group
   g is (64*(g//2), 32*g), delta tile (32g, 32g). 2g == 4*(g//2)+2*(g%2).
"""

import sys

import numpy as np

try:
    import concourse.bass as bass
except ImportError:  # fresh grading dir: point at the in-container repo
    for p in ("/opt/trn_rl_repo", "/root/.axon_site/_ro/trn_rl_repo"):
        if p not in sys.path:
            sys.path.insert(0, p)
    import concourse.bass as bass

from contextlib import ExitStack

import concourse.bacc as bacc
import concourse.tile as tile
from concourse import mybir
from concourse.bass_utils import run_bass_kernel_spmd

NB, B, C, H, W = 32, 4, 64, 80, 80
HPC = H // 2  # h rows per core (8 cores = 4 batches x 2 h-halves)
CHH = 8  # chunk height (h rows)
NHB = HPC // CHH  # 5
CW = 16  # chunk width (w cols) = supergroup width
NWB = W // CW  # 5
NCHUNK = NHB * NWB  # 25 chunks, 2 supergroups each
F16 = mybir.dt.float16
F32 = mybir.dt.float32

_CACHE = {}


def build_nc():
    nc = bacc.Bacc()
    q = nc.declare_dram_parameter("q", [NCHUNK, 128, NB, 4 * CW], F16, isOutput=False)
    k = nc.declare_dram_parameter("k", [NCHUNK, 128, NB, 4 * CW], F16, isOutput=False)
    v = nc.declare_dram_parameter("v", [NCHUNK, 128, C, 2 * CW], F16, isOutput=False)
    o = nc.declare_dram_parameter("o", [NCHUNK, 128, C, 2 * CW], F16, isOutput=True)
    qv, kv, vv, ov = q[:], k[:], v[:], o[:]

    with tile.TileContext(nc) as tc, ExitStack() as ctx:
        io = ctx.enter_context(tc.tile_pool(name="io", bufs=4))
        sm = ctx.enter_context(tc.tile_pool(name="sm", bufs=3))
        ep = ctx.enter_context(tc.tile_pool(name="ep", bufs=2, space="PSUM"))
        op = ctx.enter_context(tc.tile_pool(name="op", bufs=2, space="PSUM"))

        chunk_tiles = {}

        def emit_chunk_dma(cb):
            tq = io.tile([128, NB, 4, CW], F16, tag="tq")
            tk = io.tile([128, NB, 4, CW], F16, tag="tk")
            tv = io.tile([128, 65, 2, CW], F16, tag="tv")
            to = io.tile([128, C, 2, CW], F16, tag="to")
            nc.sync.dma_start(out=tq.rearrange("p i hw w -> p i (hw w)"), in_=qv[cb])
            nc.sync.dma_start(out=tk.rearrange("p j hw w -> p j (hw w)"), in_=kv[cb])
            nc.sync.dma_start(
                out=tv.rearrange("p c r w -> p c (r w)")[:, 0:64], in_=vv[cb]
            )
            nc.gpsimd.memset(tv[:, 64], 1.0)
            chunk_tiles[cb] = (tq, tk, tv, to)

        def emit_chunk_out(cb):
            (_, _, _, to) = chunk_tiles.pop(cb)
            nc.sync.dma_start(out=ov[cb], in_=to.rearrange("p c r w -> p c (r w)"))

        def emit_scores(sg):
            cb, r = sg["cb"], sg["r"]
            tq, tk, _, _ = chunk_tiles[cb]
            eb = ep.tile([128, 512], F32, tag="eb")
            for u in range(CW):
                for g in range(4):
                    s, hw = g // 2, 2 * (g % 2) + r
                    nc.tensor.matmul(
                        out=eb[32 * g : 32 * g + 32, 32 * u : 32 * u + 32],
                        lhsT=tk[64 * s : 64 * s + 64, :, hw, u],
                        rhs=tq[64 * s : 64 * s + 64, :, hw, u],
                        start=True,
                        stop=True,
                        tile_position=(64 * s, 32 * g),
                    )
            sg["eb"] = eb

        def emit_softmax_delta(sg):
            cb, r = sg["cb"], sg["r"]
            _, _, tv, to = chunk_tiles[cb]
            eb = sg.pop("eb")
            E = sm.tile([128, 512], F16, tag="E")
            nc.scalar.activation(E, eb, mybir.ActivationFunctionType.Exp, scale=0.125)
            ob0 = op.tile([128, 462], F32, tag="ob0")
            ob1 = op.tile([128, 462], F32, tag="ob1")
            ob2 = op.tile([128, 132], F32, tag="ob2")
            obs = [ob0, ob1, ob2]
            for u in range(CW):
                ob, uu = obs[min(u // 7, 2)], u - 7 * min(u // 7, 2)
                for g in range(4):
                    nc.tensor.matmul(
                        out=ob[32 * g : 32 * g + 32, 66 * uu : 66 * uu + 65],
                        lhsT=E[32 * g : 32 * g + 32, 32 * u : 32 * u + 32],
                        rhs=tv[32 * g : 32 * g + 32, :, r, u],
                        start=True,
                        stop=True,
                        tile_position=(32 * g, 32 * g),
                    )
            # normalize + evacuate: out inner dim is w (unit-stride fp16
            # writes); the strided side is the PSUM read (c step 1, u step 66).
            rden = sm.tile([128, 16], F32, tag="rden")
            for bi, nb in ((0, 7), (1, 7), (2, 2)):
                obv = obs[bi][:, 0 : 66 * nb].rearrange("p (u c) -> p c u", c=66)
                u0 = 7 * bi
                nc.vector.reciprocal(rden[:, u0 : u0 + nb], obv[:, 64])
                nc.vector.tensor_mul(
                    to[:, :, r, u0 : u0 + nb],
                    obv[:, 0:64],
                    rden[:, u0 : u0 + nb].unsqueeze(1).broadcast_to((128, 64, nb)),
                )

        sgs = [{"cb": cb, "r": r} for cb in range(NCHUNK) for r in range(2)]
        emit_chunk_dma(0)
        emit_chunk_dma(1)
        emit_chunk_dma(2)
        pending = None
        for sg in sgs:
            if sg["r"] == 0 and sg["cb"] + 3 < NCHUNK:
                emit_chunk_dma(sg["cb"] + 3)
            emit_scores(sg)
            if pending is not None:
                emit_softmax_delta(pending)
                if pending["r"] == 1:
                    emit_chunk_out(pending["cb"])
            pending = sg
        emit_softmax_delta(pending)
        emit_chunk_out(pending["cb"])
    nc.compile()
    return nc


def _get_nc():
    if "nc" not in _CACHE:
        _CACHE["nc"] = build_nc()
    return _CACHE["nc"]


def _pack_qk(a):
    # [32, 64, 40, 80] fp32 -> [25=(hb wb), 128=(s c), 32 i, 64=(hw w')] fp16
    t = (
        np.asarray(a)
        .reshape(NB, C, NHB, 2, 4, NWB, CW)
        .transpose(2, 5, 3, 1, 0, 4, 6)
    )
    return t.astype(np.float16).reshape(NCHUNK, 128, NB, 4 * CW)


def _pack_v(a):
    # [32, 64, 40, 80] fp32 -> [25, 128=(g j), 64 c, 32=(r w')] fp16
    t = (
        np.asarray(a)
        .reshape(NB, C, NHB, 4, 2, NWB, CW)
        .transpose(2, 5, 3, 0, 1, 4, 6)
    )
    return t.astype(np.float16).reshape(NCHUNK, 128, C, 2 * CW)


def _unpack_o(oh):
    # [25, 128=(g i), 64 c, 32=(r w')] fp16 -> [32, 64, 40, 80] fp32
    t = oh.reshape(NHB, NWB, 4, NB, C, 2, CW).astype(np.float32)
    return t.transpose(3, 4, 0, 2, 5, 1, 6).reshape(NB, C, HPC, W)


def kernel(q_big, k_big, v_big, **run_kwargs):
    nc = _get_nc()
    in_maps = []
    for core in range(8):
        b, h0 = core // 2, HPC * (core % 2)
        sl = np.s_[:, b, :, h0 : h0 + HPC, :]
        in_maps.append(
            {
                "q": _pack_qk(q_big[sl]),
                "k": _pack_qk(k_big[sl]),
                "v": _pack_v(v_big[sl]),
            }
        )
    res = run_bass_kernel_spmd(nc, in_maps, list(range(8)), **run_kwargs)
    out = np.empty((NB, B, C, H, W), np.float32)
    for core in range(8):
        b, h0 = core // 2, HPC * (core % 2)
        out[:, b, :, h0 : h0 + HPC, :] = _unpack_o(res.results[core]["o"])
    if run_kwargs:
        kernel.last_results = res
    return out
